# revision 15
# baseline (speedup 1.0000x reference)
"""Trainium2 Bass kernel for MeshNodeBlock (GNN message passing).

  agg = segment_sum(edge_feats, dst_idx, N)        # [N, D]
  cat = concat([agg, node_feats], -1)              # [N, 2D]
  h   = LN(silu(cat@W1+b1)@W2+b2) * g + b + node_feats
  returns (edge_feats, h)

Strategy (8 NeuronCores, no collectives):
  - Host sorts edges by dst and buckets them into 80 node-tiles of 128
    nodes (10240 padded nodes); each core owns 10 consecutive node tiles
    and receives exactly the edges destined to its nodes, padded per tile
    to a common E_TILE so the SPMD program is identical across cores.
  - Device: for each node tile, accumulate aggT[d, n] in PSUM over
    128-edge chunks via matmul(lhsT=edge_chunk[128e, 128d] (bf16),
    rhs=onehot[128e, 128n] (bf16)).  The one-hot is built on the fly by
    comparing the chunk's relative dst indices against an iota row
    (alternating VectorE / GpSimd so neither is the bottleneck).
  - MLP runs transposition-free: h1T[j, n] = sum_k W1[k, j].T @ catT[k, n]
    with catT = [aggT ; node_featsT] (node_featsT pre-transposed on host),
    SiLU+bias on ScalarE (bias is per-partition in this layout), then
    h2[n, d] = sum_j h1sT[j, n].T @ W2[j, d] (+ b2 via a K=1 matmul), so
    LayerNorm sees nodes on partitions / features on the free axis.
  - Edges are cast to bf16 on host: halves the HBM traffic (the memory
    roofline for this problem) while PSUM accumulation stays fp32.
"""

import sys

for _p in ("/opt/trn_rl_repo",):
    if _p not in sys.path:
        sys.path.insert(0, _p)

import ml_dtypes
import numpy as np

import concourse.bass as bass
import concourse.bacc as bacc
import concourse.tile as tile
from concourse import mybir
from concourse._compat import with_exitstack
from concourse.bass_utils import run_bass_kernel_spmd

N_NODES = 10000
N_EDGES = 320000
D = 256
HID = 256
NCORES = 8
P = 128
TPC = 10                      # node tiles per core
NT = NCORES * TPC             # 80 node tiles total
N_PAD = NT * P                # 10240 padded nodes
LOAD_CH = 4                   # edge chunks per DMA (256 KiB)

BF16 = mybir.dt.bfloat16
F32 = mybir.dt.float32
I32 = mybir.dt.int32
ALU = mybir.AluOpType
ACTF = mybir.ActivationFunctionType


@with_exitstack
def _build(ctx, tc, CH, sim_compat=False, repeat=1):
    """Emit the per-core program. CH = 128-edge chunks per node tile.

    sim_compat decomposes Silu (unimplemented in CoreSim) into
    Sigmoid + multiply; hardware uses the single fused Silu ACT op.
    """
    nc = tc.nc
    CH_TOT = TPC * CH
    ROWS = CH_TOT * P

    ef_d = nc.declare_dram_parameter("ef", [ROWS, D], BF16, isOutput=False)
    dst_d = nc.declare_dram_parameter("dstrel", [P, CH_TOT], F32, isOutput=False)
    nfT_d = nc.declare_dram_parameter("nfT", [D, TPC * P], BF16, isOutput=False)
    nfr_d = nc.declare_dram_parameter("nfres", [TPC * P, D], F32, isOutput=False)
    w1_d = nc.declare_dram_parameter("w1", [2 * D, HID], BF16, isOutput=False)
    w2_d = nc.declare_dram_parameter("w2", [HID, D], BF16, isOutput=False)
    b1_d = nc.declare_dram_parameter("b1", [P, 2], F32, isOutput=False)
    b2_d = nc.declare_dram_parameter("b2", [1, D], BF16, isOutput=False)
    g_d = nc.declare_dram_parameter("gb", [P, D], F32, isOutput=False)
    out_d = nc.declare_dram_parameter("out", [TPC * P, D], F32, isOutput=True)

    const = ctx.enter_context(tc.tile_pool(name="const", bufs=1))
    dst_sb = const.tile([P, CH_TOT], F32)
    nc.sync.dma_start(dst_sb[:], dst_d[:])
    nfT_sb = const.tile([P, 2, TPC * P], BF16)
    w1_sb = const.tile([P, 4, HID], BF16)
    for k in range(2):
        nc.sync.dma_start(nfT_sb[:, k, :], nfT_d[k * P : (k + 1) * P, :])
    for k in range(4):
        nc.sync.dma_start(w1_sb[:, k, :], w1_d[k * P : (k + 1) * P, :])
    w2_sb = const.tile([P, 2, D], BF16)
    for k in range(2):
        nc.sync.dma_start(w2_sb[:, k, :], w2_d[k * P : (k + 1) * P, :])
    b1_sb = const.tile([P, 2], F32)
    nc.sync.dma_start(b1_sb[:], b1_d[:])
    b2_sb = const.tile([1, D], BF16)
    nc.sync.dma_start(b2_sb[:], b2_d[:])
    g_sb = const.tile([P, D], F32)
    nc.sync.dma_start(g_sb[:], g_d[:])
    ones_sb = const.tile([1, P], BF16)
    nc.vector.memset(ones_sb[:], 1.0)
    eps_sb = const.tile([P, 1], F32)
    nc.vector.memset(eps_sb[:], 1e-5)
    iota_sb = const.tile([P, P], F32)
    nc.gpsimd.iota(
        iota_sb[:], pattern=[[1, P]], base=0, channel_multiplier=0,
        allow_small_or_imprecise_dtypes=True,
    )

    efp = ctx.enter_context(tc.tile_pool(name="efp", bufs=6))
    ohp = ctx.enter_context(tc.tile_pool(name="ohp", bufs=8))
    psA = ctx.enter_context(tc.tile_pool(name="psA", bufs=2, space="PSUM"))
    psB = ctx.enter_context(tc.tile_pool(name="psB", bufs=2, space="PSUM"))
    sb = ctx.enter_context(tc.tile_pool(name="sb", bufs=3))
    outp = ctx.enter_context(tc.tile_pool(name="outp", bufs=3))

    for t in [t for _ in range(repeat) for t in range(TPC)]:
        # ---- segment-sum: aggT[d, n] over this tile's edge chunks ----
        aggT0 = psA.tile([P, P], F32, tag="aggT0")
        aggT1 = psA.tile([P, P], F32, tag="aggT1")
        ef = None
        for c in range(CH):
            if c % LOAD_CH == 0:
                nch = min(LOAD_CH, CH - c)
                ef = efp.tile([P, LOAD_CH, D], BF16, tag="ef")
                src = ef_d[(t * CH + c) * P : (t * CH + c + nch) * P, :]
                nc.sync.dma_start(
                    ef[:, :nch, :], src.rearrange("(c p) d -> p c d", p=P)
                )
            ci = c % LOAD_CH
            oh = ohp.tile([P, P], BF16, tag="oh")
            nc.vector.tensor_tensor(
                oh[:], iota_sb[:],
                dst_sb[:, t * CH + c : t * CH + c + 1].to_broadcast((P, P)),
                ALU.is_equal,
            )
            nc.tensor.matmul(
                aggT0[:], ef[:, ci, 0:P], oh[:],
                start=(c == 0), stop=(c == CH - 1),
            )
            nc.tensor.matmul(
                aggT1[:], ef[:, ci, P:D], oh[:],
                start=(c == 0), stop=(c == CH - 1),
            )

        # ---- MLP: h1T = silu(W1.T @ catT + b1); h2 = h1sT.T @ W2 + b2 ----
        aggT_bf = sb.tile([P, 2, P], BF16, tag="aggTbf")
        nc.scalar.copy(aggT_bf[:, 0, :], aggT0[:])
        nc.scalar.copy(aggT_bf[:, 1, :], aggT1[:])
        h1sT = sb.tile([P, 2, P], BF16, tag="h1sT")
        for jh in range(2):
            h1T = psB.tile([P, P], F32, tag="h1T")
            for k in range(4):
                rhs = (
                    aggT_bf[:, k, :]
                    if k < 2
                    else nfT_sb[:, k - 2, t * P : (t + 1) * P]
                )
                nc.tensor.matmul(
                    h1T[:], w1_sb[:, k, jh * P : (jh + 1) * P], rhs,
                    start=(k == 0), stop=(k == 3),
                )
            if sim_compat:
                xb = sb.tile([P, P], F32, tag="xb")
                nc.scalar.activation(
                    xb[:], h1T[:], ACTF.Identity, bias=b1_sb[:, jh : jh + 1]
                )
                sg = sb.tile([P, P], F32, tag="sg")
                nc.scalar.activation(
                    sg[:], h1T[:], ACTF.Sigmoid, bias=b1_sb[:, jh : jh + 1]
                )
                nc.vector.tensor_mul(h1sT[:, jh, :], xb[:], sg[:])
            else:
                nc.scalar.activation(
                    h1sT[:, jh, :], h1T[:], ACTF.Silu, bias=b1_sb[:, jh : jh + 1]
                )
        h2 = psB.tile([P, D], F32, tag="h2")
        nc.tensor.matmul(h2[:], ones_sb[:], b2_sb[:], start=True, stop=False)
        nc.tensor.matmul(h2[:], h1sT[:, 0, :], w2_sb[:, 0, :], start=False, stop=False)
        nc.tensor.matmul(h2[:], h1sT[:, 1, :], w2_sb[:, 1, :], start=False, stop=True)

        # ---- LayerNorm (+ residual pre-folded with ln_b on host) ----
        nfr = outp.tile([P, D], F32, tag="nfr")
        nc.sync.dma_start(nfr[:], nfr_d[t * P : (t + 1) * P, :])
        musum = sb.tile([P, 1], F32, tag="musum")
        nc.vector.tensor_reduce(musum[:], h2[:], mybir.AxisListType.X, ALU.add)
        negmu = sb.tile([P, 1], F32, tag="negmu")
        nc.scalar.mul(negmu[:], musum[:], -1.0 / D)
        sq = sb.tile([P, D], F32, tag="sq")
        sumsq = sb.tile([P, 1], F32, tag="sumsq")
        nc.scalar.activation(
            sq[:], h2[:], ACTF.Square, bias=negmu[:], accum_out=sumsq[:]
        )
        std = sb.tile([P, 1], F32, tag="std")
        nc.scalar.activation(std[:], sumsq[:], ACTF.Sqrt, bias=eps_sb[:], scale=1.0 / D)
        rstd = sb.tile([P, 1], F32, tag="rstd")
        nc.vector.reciprocal(rstd[:], std[:])
        xcs = sb.tile([P, D], F32, tag="xcs")
        nc.vector.tensor_scalar(
            xcs[:], h2[:], negmu[:], rstd[:], ALU.add, ALU.mult
        )
        out_t = outp.tile([P, D], F32, tag="out")
        nc.vector.tensor_mul(xcs[:], xcs[:], g_sb[:])
        nc.vector.tensor_add(out_t[:], xcs[:], nfr[:])
        nc.sync.dma_start(out_d[t * P : (t + 1) * P, :], out_t[:])


_PROG_CACHE: dict = {}


def _get_program(CH, sim_compat=False, repeat=1):
    key = (CH, sim_compat, repeat)
    if key not in _PROG_CACHE:
        nc = bacc.Bacc("TRN2", debug=False)
        with tile.TileContext(nc) as tc:
            _build(tc, CH, sim_compat=sim_compat, repeat=repeat)
        nc.compile()
        _PROG_CACHE[key] = nc
    return _PROG_CACHE[key]


LAST_RESULTS = None  # BassKernelResults of the most recent run (for test.py)


def _prep(edge_feats, node_feats, dst_idx, W1, b1, W2, b2, ln_g, ln_b):
    """Host-side shard/pad/transpose. Returns (in_maps, CH)."""
    edge_feats = np.asarray(edge_feats)
    node_feats = np.asarray(node_feats, dtype=np.float32)
    dst = np.asarray(dst_idx).astype(np.int64)

    # ---- host: sort edges by dst, bucket into per-tile padded arrays ----
    order = np.argsort(dst, kind="stable")
    dst_s = dst[order].astype(np.int32)
    ef_s = edge_feats.astype(np.float32)[order].astype(ml_dtypes.bfloat16)
    tile_id = dst_s // P
    counts = np.bincount(tile_id, minlength=NT)
    starts = np.zeros(NT + 1, np.int64)
    starts[1:] = np.cumsum(counts)
    E_TILE = max(int(np.ceil(counts.max() / P) * P), P)
    CH = E_TILE // P
    CH_TOT = TPC * CH
    ROWS = CH_TOT * P

    ef_pad = np.zeros((NCORES, ROWS, D), ml_dtypes.bfloat16)
    dstrel = np.full((NCORES, ROWS), -1, np.float32)
    for t in range(NT):
        c, lt = divmod(t, TPC)
        s, e = int(starts[t]), int(starts[t + 1])
        n = e - s
        ef_pad[c, lt * E_TILE : lt * E_TILE + n] = ef_s[s:e]
        dstrel[c, lt * E_TILE : lt * E_TILE + n] = (dst_s[s:e] - t * P).astype(np.float32)
    dstrel_T = np.ascontiguousarray(
        dstrel.reshape(NCORES, CH_TOT, P).transpose(0, 2, 1)
    )

    nf = np.zeros((N_PAD, D), np.float32)
    nf[:N_NODES] = node_feats
    nf_res = (nf + np.asarray(ln_b, np.float32)[None, :]).reshape(NCORES, TPC * P, D)
    nfT = np.ascontiguousarray(
        nf.reshape(NCORES, TPC * P, D).transpose(0, 2, 1)
    ).astype(ml_dtypes.bfloat16)

    w1 = np.asarray(W1, np.float32).astype(ml_dtypes.bfloat16)
    w2 = np.asarray(W2, np.float32).astype(ml_dtypes.bfloat16)
    b1p = np.ascontiguousarray(np.asarray(b1, np.float32).reshape(2, P).T)
    b2r = np.asarray(b2, np.float32).astype(ml_dtypes.bfloat16).reshape(1, D)
    gb = np.ascontiguousarray(
        np.broadcast_to(np.asarray(ln_g, np.float32), (P, D))
    )

    in_maps = [
        {
            "ef": ef_pad[c],
            "dstrel": dstrel_T[c],
            "nfT": nfT[c],
            "nfres": nf_res[c],
            "w1": w1,
            "w2": w2,
            "b1": b1p,
            "b2": b2r,
            "gb": gb,
        }
        for c in range(NCORES)
    ]
    return in_maps, CH


def kernel(edge_feats, node_feats, dst_idx, W1, b1, W2, b2, ln_g, ln_b,
           _trace=False, _trace_kwargs=None):
    global LAST_RESULTS
    edge_feats = np.asarray(edge_feats)
    in_maps, CH = _prep(
        edge_feats, node_feats, dst_idx, W1, b1, W2, b2, ln_g, ln_b
    )
    nc = _get_program(CH)
    res = run_bass_kernel_spmd(
        nc, in_maps, core_ids=list(range(NCORES)),
        trace=_trace, **(_trace_kwargs or {}),
    )
    LAST_RESULTS = res

    out = np.concatenate([res.results[c]["out"] for c in range(NCORES)], axis=0)
    node_new = np.ascontiguousarray(out[:N_NODES]).astype(np.float32)
    return (edge_feats, node_new)


if __name__ == "__main__":
    rng = np.random.default_rng(0)
    E, N = 4096, N_NODES
    print("smoke test with reduced edges not supported; use test.py")


# revision 21
# speedup vs baseline: 1.9344x; 1.9344x over previous
"""Trainium2 Bass kernel for MeshNodeBlock (GNN message passing).

  agg = segment_sum(edge_feats, dst_idx, N)        # [N, D]
  cat = concat([agg, node_feats], -1)              # [N, 2D]
  h   = LN(silu(cat@W1+b1)@W2+b2) * g + b + node_feats
  returns (edge_feats, h)

Strategy (8 NeuronCores, no collectives):
  - Host sorts edges by dst and buckets them into 80 node-tiles of 128
    nodes (10240 padded nodes); each core owns 10 consecutive node tiles
    and receives exactly the edges destined to its nodes, padded per tile
    to a common E_TILE so the SPMD program is identical across cores.
  - Device: for each node tile, accumulate aggT[d, n] in PSUM over
    128-edge chunks via matmul(lhsT=edge_chunk[128e, 128d] (bf16),
    rhs=onehot[128e, 128n] (bf16)).  The one-hot is built on the fly by
    comparing the chunk's relative dst indices against an iota row
    (alternating VectorE / GpSimd so neither is the bottleneck).
  - MLP runs transposition-free: h1T[j, n] = sum_k W1[k, j].T @ catT[k, n]
    with catT = [aggT ; node_featsT] (node_featsT pre-transposed on host),
    SiLU+bias on ScalarE (bias is per-partition in this layout), then
    h2[n, d] = sum_j h1sT[j, n].T @ W2[j, d] (+ b2 via a K=1 matmul), so
    LayerNorm sees nodes on partitions / features on the free axis.
  - Edges are cast to bf16 on host: halves the HBM traffic (the memory
    roofline for this problem) while PSUM accumulation stays fp32.
"""

import sys

for _p in ("/opt/trn_rl_repo",):
    if _p not in sys.path:
        sys.path.insert(0, _p)

import ml_dtypes
import numpy as np

import concourse.bass as bass
import concourse.bacc as bacc
import concourse.tile as tile
from concourse import mybir
from concourse._compat import with_exitstack
from concourse.bass_utils import run_bass_kernel_spmd

N_NODES = 10000
N_EDGES = 320000
D = 256
HID = 256
NCORES = 8
P = 128
TPC = 10                      # node tiles per core
NT = NCORES * TPC             # 80 node tiles total
N_PAD = NT * P                # 10240 padded nodes
LOAD_CH = 4                   # edge chunks per DMA (256 KiB)

BF16 = mybir.dt.bfloat16
F32 = mybir.dt.float32
I32 = mybir.dt.int32
ALU = mybir.AluOpType
ACTF = mybir.ActivationFunctionType


@with_exitstack
def _build(ctx, tc, CH, sim_compat=False, repeat=1):
    """Emit the per-core program. CH = 128-edge chunks per node tile.

    sim_compat decomposes Silu (unimplemented in CoreSim) into
    Sigmoid + multiply; hardware uses the single fused Silu ACT op.
    """
    nc = tc.nc
    CH_TOT = TPC * CH
    ROWS = CH_TOT * P

    ef_d = nc.declare_dram_parameter("ef", [ROWS, D], BF16, isOutput=False)
    dst_d = nc.declare_dram_parameter("dstrel", [P, CH_TOT], BF16, isOutput=False)
    nfT_d = nc.declare_dram_parameter("nfT", [D, TPC * P], BF16, isOutput=False)
    nfr_d = nc.declare_dram_parameter("nfres", [TPC * P, D], F32, isOutput=False)
    w1_d = nc.declare_dram_parameter("w1", [2 * D, HID], BF16, isOutput=False)
    w2_d = nc.declare_dram_parameter("w2", [HID, D], BF16, isOutput=False)
    b1_d = nc.declare_dram_parameter("b1", [P, 2], F32, isOutput=False)
    b2_d = nc.declare_dram_parameter("b2", [1, D], BF16, isOutput=False)
    g_d = nc.declare_dram_parameter("gb", [P, D], F32, isOutput=False)
    id_d = nc.declare_dram_parameter("ident", [P, P], BF16, isOutput=False)
    out_d = nc.declare_dram_parameter("out", [TPC * P, D], F32, isOutput=True)

    const = ctx.enter_context(tc.tile_pool(name="const", bufs=1))
    dst_sb = const.tile([P, CH_TOT], BF16)
    nc.sync.dma_start(dst_sb[:], dst_d[:])
    nfT_sb = const.tile([P, 2, TPC * P], BF16)
    w1_sb = const.tile([P, 4, HID], BF16)
    for k in range(2):
        nc.sync.dma_start(nfT_sb[:, k, :], nfT_d[k * P : (k + 1) * P, :])
    for k in range(4):
        nc.sync.dma_start(w1_sb[:, k, :], w1_d[k * P : (k + 1) * P, :])
    w2_sb = const.tile([P, 2, D], BF16)
    for k in range(2):
        nc.sync.dma_start(w2_sb[:, k, :], w2_d[k * P : (k + 1) * P, :])
    b1_sb = const.tile([P, 2], F32)
    nc.sync.dma_start(b1_sb[:], b1_d[:])
    b2_sb = const.tile([1, D], BF16)
    nc.sync.dma_start(b2_sb[:], b2_d[:])
    g_sb = const.tile([P, D], F32)
    nc.sync.dma_start(g_sb[:], g_d[:])
    id_sb = const.tile([P, P], BF16)
    nc.sync.dma_start(id_sb[:], id_d[:])
    ones_sb = const.tile([1, P], BF16)
    nc.vector.memset(ones_sb[:], 1.0)
    eps_sb = const.tile([P, 1], F32)
    nc.vector.memset(eps_sb[:], 1e-5)
    iw_sb = const.tile([P, CH, P], BF16)
    nc.gpsimd.iota(
        iw_sb[:], pattern=[[0, CH], [1, P]], base=0, channel_multiplier=0,
        allow_small_or_imprecise_dtypes=True,
    )

    efp = ctx.enter_context(tc.tile_pool(name="efp", bufs=3))
    ohp = ctx.enter_context(tc.tile_pool(name="ohp", bufs=2))
    psA = ctx.enter_context(tc.tile_pool(name="psA", bufs=2, space="PSUM"))
    psB = ctx.enter_context(tc.tile_pool(name="psB", bufs=2, space="PSUM"))
    sb = ctx.enter_context(tc.tile_pool(name="sb", bufs=3))
    outp = ctx.enter_context(tc.tile_pool(name="outp", bufs=3))

    def tile_body(t):
        # ---- segment-sum: agg[n, d] over this tile's edge chunks ----
        agg = psA.tile([P, D], F32, tag="agg")
        ef = efp.tile([P, CH, D], BF16, tag="ef")
        src = ef_d[t * CH * P : (t + 1) * CH * P, :]
        nc.sync.dma_start(ef[:], src.rearrange("(c p) d -> p c d", p=P))
        ohw = ohp.tile([P, CH, P], BF16, tag="oh")
        nc.vector.tensor_tensor(
            ohw[:], iw_sb[:],
            dst_sb[:, t * CH : (t + 1) * CH].to_broadcast((P, CH, P)),
            ALU.is_equal,
        )
        for c in range(CH):
            nc.tensor.matmul(
                agg[:], ohw[:, c, :], ef[:, c, :],
                start=(c == 0), stop=(c == CH - 1),
            )

        # ---- transpose agg -> aggT (PE), cast to bf16 ----
        agg_bf = sb.tile([P, D], BF16, tag="aggbf")
        nc.scalar.copy(agg_bf[:], agg[:])
        aggT_ps = psA.tile([P, 2, P], BF16, tag="aggTps")
        nc.tensor.transpose(aggT_ps[:, 0, :], agg_bf[:, 0:P], id_sb[:])
        nc.tensor.transpose(aggT_ps[:, 1, :], agg_bf[:, P:D], id_sb[:])

        # ---- MLP: h1T = silu(W1.T @ catT + b1); h2 = h1sT.T @ W2 + b2 ----
        aggT_bf = sb.tile([P, 2, P], BF16, tag="aggTbf")
        nc.scalar.copy(aggT_bf[:], aggT_ps[:])
        h1sT = sb.tile([P, 2, P], BF16, tag="h1sT")
        for jh in range(2):
            h1T = psB.tile([P, P], F32, tag="h1T")
            for k in range(4):
                rhs = (
                    aggT_bf[:, k, :]
                    if k < 2
                    else nfT_sb[:, k - 2, t * P : (t + 1) * P]
                )
                nc.tensor.matmul(
                    h1T[:], w1_sb[:, k, jh * P : (jh + 1) * P], rhs,
                    start=(k == 0), stop=(k == 3),
                )
            if sim_compat:
                xb = sb.tile([P, P], F32, tag="xb")
                nc.scalar.activation(
                    xb[:], h1T[:], ACTF.Identity, bias=b1_sb[:, jh : jh + 1]
                )
                sg = sb.tile([P, P], F32, tag="sg")
                nc.scalar.activation(
                    sg[:], h1T[:], ACTF.Sigmoid, bias=b1_sb[:, jh : jh + 1]
                )
                nc.vector.tensor_mul(h1sT[:, jh, :], xb[:], sg[:])
            else:
                nc.scalar.activation(
                    h1sT[:, jh, :], h1T[:], ACTF.Silu, bias=b1_sb[:, jh : jh + 1]
                )
        h2 = psB.tile([P, D], F32, tag="h2")
        nc.tensor.matmul(h2[:], ones_sb[:], b2_sb[:], start=True, stop=False)
        nc.tensor.matmul(h2[:], h1sT[:, 0, :], w2_sb[:, 0, :], start=False, stop=False)
        nc.tensor.matmul(h2[:], h1sT[:, 1, :], w2_sb[:, 1, :], start=False, stop=True)

        # ---- LayerNorm (+ residual pre-folded with ln_b on host) ----
        nfr = outp.tile([P, D], F32, tag="nfr")
        nc.sync.dma_start(nfr[:], nfr_d[t * P : (t + 1) * P, :])
        musum = sb.tile([P, 1], F32, tag="musum")
        nc.vector.tensor_reduce(musum[:], h2[:], mybir.AxisListType.X, ALU.add)
        negmu = sb.tile([P, 1], F32, tag="negmu")
        nc.vector.tensor_scalar_mul(negmu[:], musum[:], -1.0 / D)
        sq = sb.tile([P, D], F32, tag="sq")
        sumsq = sb.tile([P, 1], F32, tag="sumsq")
        nc.scalar.activation(
            sq[:], h2[:], ACTF.Square, bias=negmu[:], accum_out=sumsq[:]
        )
        std = sb.tile([P, 1], F32, tag="std")
        nc.scalar.activation(std[:], sumsq[:], ACTF.Sqrt, bias=eps_sb[:], scale=1.0 / D)
        xc = sb.tile([P, D], F32, tag="xc")
        nc.vector.tensor_scalar(xc[:], h2[:], negmu[:], None, ALU.add)
        xcn = sb.tile([P, D], F32, tag="xcn")
        nc.gpsimd.normalize_recip(xcn[:], xc[:], std[:])
        out_t = outp.tile([P, D], F32, tag="out")
        nc.vector.tensor_mul(xcn[:], xcn[:], g_sb[:])
        nc.vector.tensor_add(out_t[:], xcn[:], nfr[:])
        nc.sync.dma_start(out_d[t * P : (t + 1) * P, :], out_t[:])

    if repeat > 1:
        # benchmark mode: run the whole 10-tile body `repeat` times inside
        # one NEFF via a Tile dynamic loop (slope between two repeat values
        # gives per-iteration HW time with dispatch overhead cancelled)
        with tc.For_i(0, repeat):
            for t in range(TPC):
                tile_body(t)
    else:
        for t in range(TPC):
            tile_body(t)


_PROG_CACHE: dict = {}


def _get_program(CH, sim_compat=False, repeat=1):
    key = (CH, sim_compat, repeat)
    if key not in _PROG_CACHE:
        nc = bacc.Bacc("TRN2", debug=False)
        with tile.TileContext(nc) as tc:
            _build(tc, CH, sim_compat=sim_compat, repeat=repeat)
        nc.compile()
        _PROG_CACHE[key] = nc
    return _PROG_CACHE[key]


LAST_RESULTS = None  # BassKernelResults of the most recent run (for test.py)


def _prep(edge_feats, node_feats, dst_idx, W1, b1, W2, b2, ln_g, ln_b):
    """Host-side shard/pad/transpose. Returns (in_maps, CH)."""
    edge_feats = np.asarray(edge_feats)
    node_feats = np.asarray(node_feats, dtype=np.float32)
    dst = np.asarray(dst_idx).astype(np.int64)

    # ---- host: sort edges by dst, bucket into per-tile padded arrays ----
    order = np.argsort(dst, kind="stable")
    dst_s = dst[order].astype(np.int32)
    ef_s = edge_feats.astype(np.float32)[order].astype(ml_dtypes.bfloat16)
    tile_id = dst_s // P
    counts = np.bincount(tile_id, minlength=NT)
    starts = np.zeros(NT + 1, np.int64)
    starts[1:] = np.cumsum(counts)
    E_TILE = max(int(np.ceil(counts.max() / P) * P), P)
    CH = E_TILE // P
    CH_TOT = TPC * CH
    ROWS = CH_TOT * P

    ef_pad = np.zeros((NCORES, ROWS, D), ml_dtypes.bfloat16)
    dstrel = np.full((NCORES, ROWS), -1, np.float32)
    for t in range(NT):
        c, lt = divmod(t, TPC)
        s, e = int(starts[t]), int(starts[t + 1])
        n = e - s
        ef_pad[c, lt * E_TILE : lt * E_TILE + n] = ef_s[s:e]
        dstrel[c, lt * E_TILE : lt * E_TILE + n] = (dst_s[s:e] - t * P).astype(np.float32)
    dstrel_T = np.ascontiguousarray(
        dstrel.reshape(NCORES, CH_TOT, P).transpose(0, 2, 1)
    ).astype(ml_dtypes.bfloat16)

    nf = np.zeros((N_PAD, D), np.float32)
    nf[:N_NODES] = node_feats
    nf_res = (nf + np.asarray(ln_b, np.float32)[None, :]).reshape(NCORES, TPC * P, D)
    nfT = np.ascontiguousarray(
        nf.reshape(NCORES, TPC * P, D).transpose(0, 2, 1)
    ).astype(ml_dtypes.bfloat16)

    w1 = np.asarray(W1, np.float32).astype(ml_dtypes.bfloat16)
    w2 = np.asarray(W2, np.float32).astype(ml_dtypes.bfloat16)
    b1p = np.ascontiguousarray(np.asarray(b1, np.float32).reshape(2, P).T)
    b2r = np.asarray(b2, np.float32).astype(ml_dtypes.bfloat16).reshape(1, D)
    gb = np.ascontiguousarray(
        np.broadcast_to(np.asarray(ln_g, np.float32), (P, D))
    )
    ident = np.eye(P, dtype=np.float32).astype(ml_dtypes.bfloat16)

    in_maps = [
        {
            "ef": ef_pad[c],
            "dstrel": dstrel_T[c],
            "nfT": nfT[c],
            "nfres": nf_res[c],
            "w1": w1,
            "w2": w2,
            "b1": b1p,
            "b2": b2r,
            "gb": gb,
            "ident": ident,
        }
        for c in range(NCORES)
    ]
    return in_maps, CH


def kernel(edge_feats, node_feats, dst_idx, W1, b1, W2, b2, ln_g, ln_b,
           _trace=False, _trace_kwargs=None):
    global LAST_RESULTS
    edge_feats = np.asarray(edge_feats)
    in_maps, CH = _prep(
        edge_feats, node_feats, dst_idx, W1, b1, W2, b2, ln_g, ln_b
    )
    nc = _get_program(CH)
    res = run_bass_kernel_spmd(
        nc, in_maps, core_ids=list(range(NCORES)),
        trace=_trace, **(_trace_kwargs or {}),
    )
    LAST_RESULTS = res

    out = np.concatenate([res.results[c]["out"] for c in range(NCORES)], axis=0)
    node_new = np.ascontiguousarray(out[:N_NODES]).astype(np.float32)
    return (edge_feats, node_new)


if __name__ == "__main__":
    rng = np.random.default_rng(0)
    E, N = 4096, N_NODES
    print("smoke test with reduced edges not supported; use test.py")
